# revision 8
# baseline (speedup 1.0000x reference)
"""Trainium2 Bass kernel for CRsAE FISTA sparse coding (nn_CRsAERandProj).

reference semantics:
    phiH = (phi @ H)[0]                       # [64, 1024]
    x2 = x[..., 0]                            # [B=4096, 64]
    FISTA for T iters:
        resid = x2 - yk @ phiH.T
        x_new = soft(yk + resid @ phiH / L, lam/L)
        yk    = x_new + beta_i * (x_new - x_old)
    returns (x_new @ phiH.T)[..., None], x_new[..., None]

Device formulation (per core, batch shard BSH=512, enc-partitioned state):
    state x_i stored as [128p, 8, BSH] (enc on partitions, batch on free dim).
    y_i = a_i*x_i - b_i*x_{i-1} is never materialized:
      * its projection phiH @ y_i^T is formed from saved projections
        q_k = phiH @ x_k^T (tiny [64, BSH] tensors)       -> "q trick"
      * its contribution to v = y + resid@phiH/L is accumulated straight
        into PSUM with scaled-identity matmuls.
    Per iteration (matmul operands bf16 for full PE rate, fp32 accumulate):
      PE  : psum_j = a*I @ x_j + (-b)*I @ xprev_j + phiHs_j^T @ r   (24 MMs)
            psum_q = sum_j phiHT_j^T @ xnew_j                        (8 MMs)
      ACT : v_j = copy(psum_j) -> bf16 SBUF
      DVE : u_j = clamp(v_j, +-lam/L); xnew_j = v_j - u_j
      DVE : r' = -(1+beta)*q + t0 ; t0' = beta'*q + x2t   ([64,BSH] smalls)
Final outputs: z^T = phiH @ x_T^T == the last q psum; x_T tiles DMA'd out.
"""

import os
import numpy as np
import ml_dtypes

B = 4096
D_IN = 64
D_ENC = 1024
NCORES = 8
BSH = B // NCORES          # 512 batch rows per core
P = 128
NTILE = D_ENC // P         # 8 enc tiles per core
L_CONST = 50.0
LAM = 0.1

# how many enc-tiles' x=v-u subtract run on DVE (rest go to GPSIMD)
DVE_SUB_TILES = int(os.environ.get("KERNEL_DVE_SUB_TILES", "8"))

_cache = {}


def _betas_f32(T):
    """beta_i = (t_i - 1)/t_{i+1} computed in fp32 like the reference."""
    one = np.float32(1.0)
    t = np.float32(1.0)
    betas = []
    for _ in range(T):
        tn = np.float32((one + np.float32(np.sqrt(one + np.float32(4.0) * t * t)))
                        / np.float32(2.0))
        betas.append(np.float32((t - one) / tn))
        t = tn
    return betas


def _id_coefs(T):
    """identity schedule: iter i>=1 uses A_i=(1+beta_{i-1})I, i>=2 also
    B_i=(-beta_{i-1})I.  Returns (coefs, idxA, idxB)."""
    betas = _betas_f32(T)
    coefs = []
    idxA = {}
    idxB = {}
    for i in range(1, T):
        idxA[i] = len(coefs)
        coefs.append(np.float32(1.0) + betas[i - 1])
    for i in range(2, T):
        idxB[i] = len(coefs)
        coefs.append(-betas[i - 1])
    return coefs, idxA, idxB


def _build(T):
    import concourse.tile as tile
    import concourse.mybir as mybir
    from concourse import bacc

    f32 = mybir.dt.float32
    bf16 = mybir.dt.bfloat16
    Alu = mybir.AluOpType
    thr = float(np.float32(LAM) / np.float32(L_CONST))

    betas = _betas_f32(T)
    coefs, idxA, idxB = _id_coefs(T)
    NID = max(len(coefs), 1)

    nc = bacc.Bacc("TRN2", target_bir_lowering=False, debug=False)

    d_x2t = nc.dram_tensor("x2t", [D_IN, BSH], f32, kind="ExternalInput").ap()
    d_pht = nc.dram_tensor("phiht", [P, NTILE, D_IN], bf16, kind="ExternalInput").ap()
    d_phs = nc.dram_tensor("phihs", [D_IN, D_ENC], bf16, kind="ExternalInput").ap()
    d_ids = nc.dram_tensor("idents", [P, NID, P], bf16, kind="ExternalInput").ap()
    d_zt = nc.dram_tensor("zt", [D_IN, BSH], f32, kind="ExternalOutput").ap()
    d_xt = nc.dram_tensor("xt", [P, NTILE, BSH], bf16, kind="ExternalOutput").ap()

    with tile.TileContext(nc) as tc:
        with (
            tc.tile_pool(name="const", bufs=1) as cpool,
            tc.tile_pool(name="state", bufs=1) as spool,
            tc.tile_pool(name="v", bufs=3) as vpool,
            tc.tile_pool(name="u", bufs=3) as upool,
            tc.tile_pool(name="psv", bufs=6, space="PSUM") as psv,
            tc.tile_pool(name="psq", bufs=2, space="PSUM") as psq,
        ):
            sb_x2t = cpool.tile([D_IN, BSH], f32)
            nc.sync.dma_start(sb_x2t[:], d_x2t[:])
            sb_pht = cpool.tile([P, NTILE, D_IN], bf16)
            for j in range(NTILE):
                nc.sync.dma_start(sb_pht[:, j, :], d_pht[:, j, :])
            sb_phs = cpool.tile([D_IN, D_ENC], bf16)
            nc.sync.dma_start(sb_phs[:], d_phs[:])
            sb_ids = cpool.tile([P, NID, P], bf16)
            for k in range(NID):
                nc.sync.dma_start(sb_ids[:, k, :], d_ids[:, k, :])
            # bf16 copy of x2t for the iter-0 matmul rhs
            sb_x2tb = cpool.tile([D_IN, BSH], bf16)
            nc.vector.tensor_copy(sb_x2tb[:], sb_x2t[:])

            xa = spool.tile([P, NTILE, BSH], bf16)
            xb = spool.tile([P, NTILE, BSH], bf16)
            t0 = [spool.tile([D_IN, BSH], f32, name=f"t0_{k}")
                  for k in range(2)]
            rr = spool.tile([D_IN, BSH], bf16)
            sb_zt = spool.tile([D_IN, BSH], f32)

            for i in range(T):
                xw = xb if i % 2 == 0 else xa   # holds x_{i-1}; receives x_{i+1}
                xc = xa if i % 2 == 0 else xb   # holds x_i (valid for i >= 1)
                r_ap = sb_x2tb if i == 0 else rr

                vps = [psv.tile([P, BSH], f32, name=f"vps_{i}_{j}", tag="vps")
                       for j in range(NTILE)]

                def emit_A(j, i=i, vps=vps, xc=xc, xw=xw):
                    if i >= 1:
                        nc.tensor.matmul(
                            vps[j][:],
                            lhsT=sb_ids[:, idxA[i], :],
                            rhs=xc[:, j, :],
                            start=True, stop=False,
                        )
                    if i >= 2:
                        nc.tensor.matmul(
                            vps[j][:],
                            lhsT=sb_ids[:, idxB[i], :],
                            rhs=xw[:, j, :],
                            start=False, stop=False,
                        )

                def emit_B(j, i=i, vps=vps, r_ap=r_ap):
                    nc.tensor.matmul(
                        vps[j][:],
                        lhsT=sb_phs[:, j * P:(j + 1) * P],
                        rhs=r_ap[:],
                        start=(i == 0), stop=True,
                    )

                def emit_C(j, i=i, vps=vps, xw=xw):
                    v = vpool.tile([P, BSH], bf16, name=f"v_{i}_{j}", tag="v")
                    nc.scalar.copy(v[:], vps[j][:])
                    u = upool.tile([P, BSH], bf16, name=f"u_{i}_{j}", tag="u")
                    nc.vector.tensor_scalar(
                        u[:], v[:], -thr, thr, Alu.max, Alu.min)
                    eng = nc.vector if j < DVE_SUB_TILES else nc.gpsimd
                    eng.scalar_tensor_tensor(
                        xw[:, j, :], v[:], 0.0, u[:], Alu.add, Alu.subtract)
                    if i == T - 1:
                        nc.sync.dma_start(d_xt[:, j, :], xw[:, j, :])

                # stagger so PE never waits on a psum slot or on r
                if i == 0:
                    for j in range(NTILE):
                        emit_B(j)
                        emit_C(j)
                else:
                    for j in range(4):
                        emit_A(j)
                    for j in range(NTILE):
                        emit_B(j)
                        if j + 4 < NTILE:
                            emit_A(j + 4)
                        emit_C(j)

                # q = phiH @ x_{i+1}^T  (accumulated over enc tiles)
                qps = psq.tile([D_IN, BSH], f32, name=f"qps_{i}", tag="qps")
                for j in range(NTILE):
                    nc.tensor.matmul(
                        qps[:],
                        lhsT=sb_pht[:, j, :],
                        rhs=xw[:, j, :],
                        start=(j == 0), stop=(j == NTILE - 1),
                    )

                if i < T - 1:
                    # r_{i+1} = -(1+beta_i)*q + t0_{i+1}
                    t0_in = sb_x2t if i == 0 else t0[(i - 1) % 2]
                    nc.vector.scalar_tensor_tensor(
                        rr[:], qps[:], -(1.0 + float(betas[i])), t0_in[:],
                        Alu.mult, Alu.add)
                    if i < T - 2:
                        # t0_{i+2} = beta_{i+1}*q + x2t
                        nc.vector.scalar_tensor_tensor(
                            t0[i % 2][:], qps[:], float(betas[i + 1]),
                            sb_x2t[:], Alu.mult, Alu.add)
                else:
                    nc.scalar.copy(sb_zt[:], qps[:])
                    nc.sync.dma_start(d_zt[:], sb_zt[:])

    nc.compile()
    return nc


def _get_nc(T):
    if T not in _cache:
        _cache[T] = _build(T)
    return _cache[T]


def _host_prep(x, H, phi, T):
    bf = ml_dtypes.bfloat16
    x = np.asarray(x, np.float32)
    H = np.asarray(H, np.float32)
    phi = np.asarray(phi, np.float32)
    phiH = (phi[0] @ H).astype(np.float32)               # [64, 1024]
    x2 = np.ascontiguousarray(x[..., 0])                 # [B, 64]

    phiHT = np.ascontiguousarray(
        phiH.T.reshape(NTILE, P, D_IN).transpose(1, 0, 2)).astype(bf)
    phiHs = np.ascontiguousarray(phiH * np.float32(1.0 / L_CONST)).astype(bf)

    coefs, _, _ = _id_coefs(T)
    NID = max(len(coefs), 1)
    idents = np.zeros((P, NID, P), np.float32)
    eye = np.eye(P, dtype=np.float32)
    for k, c in enumerate(coefs):
        idents[:, k, :] = eye * c
    idents = idents.astype(bf)

    in_maps = []
    for c in range(NCORES):
        x2t_c = np.ascontiguousarray(x2[c * BSH:(c + 1) * BSH].T)  # [64, 512]
        in_maps.append({
            "x2t": x2t_c,
            "phiht": phiHT,
            "phihs": phiHs,
            "idents": idents,
        })
    return in_maps


def _gather(results):
    z = np.empty((B, D_IN), np.float32)
    xn = np.empty((B, D_ENC), np.float32)
    for c, res in enumerate(results):
        zt = res["zt"]                          # [64, 512]
        xt = np.asarray(res["xt"], np.float32)  # [128, 8, 512] bf16->f32
        z[c * BSH:(c + 1) * BSH] = zt.T
        xn[c * BSH:(c + 1) * BSH] = (
            xt.transpose(1, 0, 2).reshape(D_ENC, BSH).T)
    return z[..., None], xn[..., None]


def run_sharded(x, H, phi, T, trace=False):
    """Run the bass kernel on 8 cores; returns ((z, x_new), BassKernelResults)."""
    from concourse.bass_utils import run_bass_kernel_spmd

    T = int(T)
    nc = _get_nc(T)
    in_maps = _host_prep(x, H, phi, T)
    res = run_bass_kernel_spmd(nc, in_maps, list(range(NCORES)), trace=trace)
    return _gather(res.results), res


def kernel(x, H, phi, T):
    out, _ = run_sharded(x, H, phi, T, trace=False)
    return out


# revision 23
# speedup vs baseline: 48.7552x; 48.7552x over previous
"""Trainium2 Bass kernel for CRsAE FISTA sparse coding (nn_CRsAERandProj).

reference semantics:
    phiH = (phi @ H)[0]                       # [64, 1024]
    x2 = x[..., 0]                            # [B=4096, 64]
    FISTA for T iters:
        resid = x2 - yk @ phiH.T
        x_new = soft(yk + resid @ phiH / L, lam/L)
        yk    = x_new + beta_i * (x_new - x_old)
    returns (x_new @ phiH.T)[..., None], x_new[..., None]

Device formulation (per core, batch shard BSH=512, enc-partitioned state):
    state x_i stored as [128p, 8, BSH] (enc on partitions, batch on free dim).
    y_i = a_i*x_i - b_i*x_{i-1} is never materialized:
      * its projection phiH @ y_i^T is formed from saved projections
        q_k = phiH @ x_k^T (tiny [64, BSH] tensors)       -> "q trick"
      * its contribution to v = y + resid@phiH/L is accumulated straight
        into PSUM with scaled-identity matmuls.
    Per iteration (matmul operands bf16 for full PE rate, fp32 accumulate):
      PE  : psum_j = a*I @ x_j + (-b)*I @ xprev_j + phiHs_j^T @ r   (24 MMs)
            psum_q = sum_j phiHT_j^T @ xnew_j                        (8 MMs)
      ACT : v_j = copy(psum_j) -> bf16 SBUF
      DVE : u_j = clamp(v_j, +-lam/L); xnew_j = v_j - u_j
      DVE : r' = -(1+beta)*q + t0 ; t0' = beta'*q + x2t   ([64,BSH] smalls)
Final outputs: z^T = phiH @ x_T^T == the last q psum; x_T tiles DMA'd out.
"""

import os
import numpy as np
import ml_dtypes

B = 4096
D_IN = 64
D_ENC = 1024
NCORES = 8
BSH = B // NCORES          # 512 batch rows per core
P = 128
NTILE = D_ENC // P         # 8 enc tiles per core
L_CONST = 50.0
LAM = 0.1

# how many enc-tiles' x=v-u subtract run on DVE (rest go to GPSIMD)
DVE_SUB_TILES = int(os.environ.get("KERNEL_DVE_SUB_TILES", "4"))

_cache = {}


def _betas_f32(T):
    """beta_i = (t_i - 1)/t_{i+1} computed in fp32 like the reference."""
    one = np.float32(1.0)
    t = np.float32(1.0)
    betas = []
    for _ in range(T):
        tn = np.float32((one + np.float32(np.sqrt(one + np.float32(4.0) * t * t)))
                        / np.float32(2.0))
        betas.append(np.float32((t - one) / tn))
        t = tn
    return betas


def _id_coefs(T):
    """identity schedule: iter i>=1 uses A_i=(1+beta_{i-1})I, i>=2 also
    B_i=(-beta_{i-1})I.  Interleaved in iteration order so the DMA stream
    delivers early iterations' matrices first.  Returns (coefs, idxA, idxB)."""
    betas = _betas_f32(T)
    coefs = []
    idxA = {}
    idxB = {}
    for i in range(1, T):
        idxA[i] = len(coefs)
        coefs.append(np.float32(1.0) + betas[i - 1])
        if i >= 2:
            idxB[i] = len(coefs)
            coefs.append(-betas[i - 1])
    return coefs, idxA, idxB


def _build(T, repeats=1):
    """repeats>1 unrolls the whole T-iteration body multiple times in one
    NEFF — used only for differential HW timing (outputs are garbage for
    r>0 starts, timing is identical per block)."""
    import concourse.tile as tile
    import concourse.mybir as mybir
    from concourse import bacc

    f32 = mybir.dt.float32
    bf16 = mybir.dt.bfloat16
    Alu = mybir.AluOpType
    thr = float(np.float32(LAM) / np.float32(L_CONST))

    betas = _betas_f32(T)
    coefs, idxA, idxB = _id_coefs(T)
    NID = max(len(coefs), 1)

    nc = bacc.Bacc("TRN2", target_bir_lowering=False, debug=False)

    d_x2t = nc.dram_tensor("x2t", [D_IN, BSH], f32, kind="ExternalInput").ap()
    d_pht = nc.dram_tensor("phiht", [P, NTILE, D_IN], bf16, kind="ExternalInput").ap()
    d_phs = nc.dram_tensor("phihs", [D_IN, D_ENC], bf16, kind="ExternalInput").ap()
    d_ids = nc.dram_tensor("idents", [P, NID, P], bf16, kind="ExternalInput").ap()
    d_zt = nc.dram_tensor("zt", [D_IN, BSH], f32, kind="ExternalOutput").ap()
    d_xt = nc.dram_tensor("xt", [P, NTILE, BSH], bf16, kind="ExternalOutput").ap()

    with tile.TileContext(nc) as tc:
        with (
            tc.tile_pool(name="const", bufs=1) as cpool,
            tc.tile_pool(name="state", bufs=1) as spool,
            tc.tile_pool(name="v", bufs=3) as vpool,
            tc.tile_pool(name="u", bufs=3) as upool,
            tc.tile_pool(name="psv", bufs=6, space="PSUM") as psv,
            tc.tile_pool(name="psq", bufs=2, space="PSUM") as psq,
        ):
            sb_x2t = cpool.tile([D_IN, BSH], f32)
            nc.sync.dma_start(sb_x2t[:], d_x2t[:])
            sb_pht = cpool.tile([P, NTILE, D_IN], bf16)
            for j in range(NTILE):
                nc.sync.dma_start(sb_pht[:, j, :], d_pht[:, j, :])
            sb_phs = cpool.tile([D_IN, D_ENC], bf16)
            nc.sync.dma_start(sb_phs[:], d_phs[:])
            sb_ids = cpool.tile([P, NID, P], bf16)
            for k in range(NID):
                nc.sync.dma_start(sb_ids[:, k, :], d_ids[:, k, :])
            # bf16 copy of x2t for the iter-0 matmul rhs
            sb_x2tb = cpool.tile([D_IN, BSH], bf16)
            nc.vector.tensor_copy(sb_x2tb[:], sb_x2t[:])

            xa = spool.tile([P, NTILE, BSH], bf16)
            xb = spool.tile([P, NTILE, BSH], bf16)
            t0 = [spool.tile([D_IN, BSH], f32, name=f"t0_{k}")
                  for k in range(2)]
            rr = spool.tile([D_IN, BSH], bf16)
            sb_zt = spool.tile([D_IN, BSH], f32)

            for i0 in range(T * repeats):
                i = i0 % T
                rep = i0 // T
                # structural iteration index: repeat blocks after the first
                # behave like iteration>=1 (state carries across, keeps the
                # timing-repeat from being dead-code-eliminated)
                si = i if rep == 0 else max(i, 1)
                xw = xb if i % 2 == 0 else xa   # holds x_{i-1}; receives x_{i+1}
                xc = xa if i % 2 == 0 else xb   # holds x_i (valid for i >= 1)
                r_ap = sb_x2tb if i0 == 0 else rr

                vps = [psv.tile([P, BSH], f32, name=f"vps_{i0}_{j}", tag="vps")
                       for j in range(NTILE)]

                def emit_A(j, si=si, vps=vps, xc=xc, xw=xw):
                    if si >= 1:
                        nc.tensor.matmul(
                            vps[j][:],
                            lhsT=sb_ids[:, idxA[si], :],
                            rhs=xc[:, j, :],
                            start=True, stop=False,
                        )
                    if si >= 2:
                        nc.tensor.matmul(
                            vps[j][:],
                            lhsT=sb_ids[:, idxB[si], :],
                            rhs=xw[:, j, :],
                            start=False, stop=False,
                        )

                def emit_B(j, i0=i0, vps=vps, r_ap=r_ap):
                    nc.tensor.matmul(
                        vps[j][:],
                        lhsT=sb_phs[:, j * P:(j + 1) * P],
                        rhs=r_ap[:],
                        start=(i0 == 0), stop=True,
                    )

                def emit_C(j, i=i, i0=i0, vps=vps, xw=xw):
                    u = upool.tile([P, BSH], bf16, name=f"u_{i0}_{j}", tag="u")
                    if j >= NTILE - 1:
                        # latency-critical tail tiles: skip the ACT copy,
                        # read PSUM directly on DVE (shortens the
                        # B -> x -> q -> r critical cycle)
                        nc.vector.tensor_scalar(
                            u[:], vps[j][:], -thr, thr, Alu.max, Alu.min)
                        nc.vector.scalar_tensor_tensor(
                            xw[:, j, :], vps[j][:], 0.0, u[:],
                            Alu.add, Alu.subtract)
                    else:
                        v = vpool.tile([P, BSH], bf16, name=f"v_{i0}_{j}", tag="v")
                        nc.scalar.copy(v[:], vps[j][:])
                        nc.vector.tensor_scalar(
                            u[:], v[:], -thr, thr, Alu.max, Alu.min)
                        if j < DVE_SUB_TILES:
                            nc.vector.scalar_tensor_tensor(
                                xw[:, j, :], v[:], 0.0, u[:],
                                Alu.add, Alu.subtract)
                        else:
                            nc.gpsimd.tensor_sub(xw[:, j, :], v[:], u[:])
                    if i == T - 1:
                        nc.sync.dma_start(d_xt[:, j, :], xw[:, j, :])

                # stagger so PE never waits on a psum slot or on r
                if si == 0:
                    for j in range(NTILE):
                        emit_B(j)
                        emit_C(j)
                else:
                    for j in range(6):
                        emit_A(j)
                    for j in range(NTILE):
                        emit_B(j)
                        if j + 6 < NTILE:
                            emit_A(j + 6)
                        emit_C(j)

                # q = phiH @ x_{i+1}^T  (accumulated over enc tiles)
                qps = psq.tile([D_IN, BSH], f32, name=f"qps_{i0}", tag="qps")
                for j in range(NTILE):
                    nc.tensor.matmul(
                        qps[:],
                        lhsT=sb_pht[:, j, :],
                        rhs=xw[:, j, :],
                        start=(j == 0), stop=(j == NTILE - 1),
                    )

                if i < T - 1 or rep < repeats - 1:
                    # r_{i+1} = -(1+beta_i)*q + t0_{i+1}
                    bi = betas[i] if i < T - 1 else betas[0]
                    t0_in = sb_x2t if i == 0 else t0[(i - 1) % 2]
                    nc.vector.scalar_tensor_tensor(
                        rr[:], qps[:], -(1.0 + float(bi)), t0_in[:],
                        Alu.mult, Alu.add)
                    if i < T - 2:
                        # t0_{i+2} = beta_{i+1}*q + x2t  (off the critical
                        # path: ACT does the scaled psum read, Pool the add)
                        qq = vpool.tile([D_IN, BSH], f32,
                                        name=f"qq_{i0}", tag="qq")
                        nc.scalar.mul(qq[:], qps[:], float(betas[i + 1]))
                        nc.gpsimd.tensor_add(t0[i % 2][:], qq[:], sb_x2t[:])
                if i == T - 1:
                    nc.scalar.copy(sb_zt[:], qps[:])
                    nc.sync.dma_start(d_zt[:], sb_zt[:])

    nc.compile()
    return nc


def _get_nc(T):
    if T not in _cache:
        _cache[T] = _build(T)
    return _cache[T]


def _host_prep(x, H, phi, T):
    bf = ml_dtypes.bfloat16
    x = np.asarray(x, np.float32)
    H = np.asarray(H, np.float32)
    phi = np.asarray(phi, np.float32)
    phiH = (phi[0] @ H).astype(np.float32)               # [64, 1024]
    x2 = np.ascontiguousarray(x[..., 0])                 # [B, 64]

    phiHT = np.ascontiguousarray(
        phiH.T.reshape(NTILE, P, D_IN).transpose(1, 0, 2)).astype(bf)
    phiHs = np.ascontiguousarray(phiH * np.float32(1.0 / L_CONST)).astype(bf)

    coefs, _, _ = _id_coefs(T)
    NID = max(len(coefs), 1)
    idents = np.zeros((P, NID, P), np.float32)
    eye = np.eye(P, dtype=np.float32)
    for k, c in enumerate(coefs):
        idents[:, k, :] = eye * c
    idents = idents.astype(bf)

    in_maps = []
    for c in range(NCORES):
        x2t_c = np.ascontiguousarray(x2[c * BSH:(c + 1) * BSH].T)  # [64, 512]
        in_maps.append({
            "x2t": x2t_c,
            "phiht": phiHT,
            "phihs": phiHs,
            "idents": idents,
        })
    return in_maps


def _gather(results):
    z = np.empty((B, D_IN), np.float32)
    xn = np.empty((B, D_ENC), np.float32)
    for c, res in enumerate(results):
        zt = res["zt"]                          # [64, 512]
        xt = np.asarray(res["xt"], np.float32)  # [128, 8, 512] bf16->f32
        z[c * BSH:(c + 1) * BSH] = zt.T
        xn[c * BSH:(c + 1) * BSH] = (
            xt.transpose(1, 0, 2).reshape(D_ENC, BSH).T)
    return z[..., None], xn[..., None]


def run_sharded(x, H, phi, T, trace=False):
    """Run the bass kernel on 8 cores; returns ((z, x_new), BassKernelResults)."""
    from concourse.bass_utils import run_bass_kernel_spmd

    T = int(T)
    nc = _get_nc(T)
    in_maps = _host_prep(x, H, phi, T)
    res = run_bass_kernel_spmd(nc, in_maps, list(range(NCORES)), trace=trace)
    return _gather(res.results), res


def kernel(x, H, phi, T):
    out, _ = run_sharded(x, H, phi, T, trace=False)
    return out
